# revision 1
# baseline (speedup 1.0000x reference)
"""Trainium2 Bass kernel for CausalTensionGraphLayer.

Math (reference factorization):
  a   = x @ w1[:D] + b1         [T, H]   (H = D/2)
  c   = x @ w1[D:]              [T, H]
  vzb = x @ wv_w + wv_b         [T, D]
  hid_w  = silu(a[t] + c[t-w-1])               (c term is 0 when t-w-1 < 0)
  tau_w  = sigmoid(hid_w @ w2 + b2)
  msg[t] = sum_w tau_w[t] * vzb[t-w-1]         (vzb -> wv_b when t-w-1 < 0)
  y      = x @ merge_w[:D] + msg @ merge_w[D:] + merge_b
  out    = LayerNorm(y) * gamma + beta

Neighbor gathers are row shifts of x, so with zero rows prepended for the
out-of-range halo the same compute path reproduces the reference exactly
(zero x rows give c = 0 and vzb = wv_b).

Sharding: data-parallel over the B*T = 8192 token rows, 1024 own tokens per
core plus a 4-row halo (zeros at batch boundaries, neighbor rows otherwise).
No collectives. Host pre-casts x/weights to bf16 and pre-transposes x so the
device works feature-major (tokens on the free axis -> shifts are free-dim
offsets).

Schedule: phase A (a+c, all token quarters) needs only x/w1 so the PE starts
while wv and the merge weights are still streaming in; phase B (vzb) overlaps
the merge-weight loads; phases C (gating) and D (merge+LN) run per quarter.
Input DMAs are split across the two HWDGE queues (sync, scalar) in the exact
order the PE consumes them.
"""

from contextlib import ExitStack

import numpy as np
import ml_dtypes

import concourse.bass as bass
import concourse.bacc as bacc
import concourse.tile as tile
from concourse import mybir
from concourse.bass_utils import run_bass_kernel_spmd

BF16 = ml_dtypes.bfloat16

B, T, D = 2, 4096, 1024
H = D // 2
W = 4
EPS = 1e-5
NCORES = 8
NTOK = (B * T) // NCORES          # 1024 own tokens per core
HALO = W                          # 4
GRID = NTOK + HALO                # 1028 (halo + own)
NQ = 4                            # token quarters per core
QT = NTOK // NQ                   # 256 own tokens per quarter
QG = QT + HALO                    # 260: shifted-grid cols per quarter
KD = D // 128                     # 8 K-chunks over D
MH = H // 128                     # 4 M-tiles over H
MD = D // 128                     # 8 M-tiles over D
NT = QT // 128                    # 2 token tiles per quarter

FP32 = mybir.dt.float32
I32 = mybir.dt.int32
BF = mybir.dt.bfloat16
AF = mybir.ActivationFunctionType
ALU = mybir.AluOpType
AX = mybir.AxisListType


def build_nc(use_gamma_beta: bool, use_merge_b: bool):
    nc = bacc.Bacc(None, target_bir_lowering=False)

    xT = nc.dram_tensor("xT", [D, GRID], BF, kind="ExternalInput")
    w1a = nc.dram_tensor("w1a", [D, H], BF, kind="ExternalInput")
    w1c = nc.dram_tensor("w1c", [D, H], BF, kind="ExternalInput")
    wv = nc.dram_tensor("wv", [D, D], BF, kind="ExternalInput")
    m1 = nc.dram_tensor("m1", [D, D], BF, kind="ExternalInput")
    m2 = nc.dram_tensor("m2", [D, D], BF, kind="ExternalInput")
    w2rep = nc.dram_tensor("w2rep", [H, 128], BF, kind="ExternalInput")
    b1r = nc.dram_tensor("b1r", [128, MH], FP32, kind="ExternalInput")
    wvbr = nc.dram_tensor("wvbr", [128, MD], FP32, kind="ExternalInput")
    b2r = nc.dram_tensor("b2r", [128, 1], FP32, kind="ExternalInput")
    if use_gamma_beta:
        gam = nc.dram_tensor("gam", [1, D], FP32, kind="ExternalInput")
        bet = nc.dram_tensor("bet", [1, D], FP32, kind="ExternalInput")
    if use_merge_b:
        mbt = nc.dram_tensor("mbt", [1, D], FP32, kind="ExternalInput")
    y = nc.dram_tensor("y", [NTOK, D], FP32, kind="ExternalOutput")

    with tile.TileContext(nc) as tc, ExitStack() as ctx:
        persist = ctx.enter_context(tc.tile_pool(name="persist", bufs=1))
        abpool = ctx.enter_context(tc.tile_pool(name="abpool", bufs=NQ))
        qpool = ctx.enter_context(tc.tile_pool(name="qpool", bufs=2))
        mpool = ctx.enter_context(tc.tile_pool(name="mpool", bufs=4))
        mpool2 = ctx.enter_context(tc.tile_pool(name="mpool2", bufs=2))
        opool = ctx.enter_context(tc.tile_pool(name="opool", bufs=3))
        ps_acc = ctx.enter_context(tc.tile_pool(name="ps_acc", bufs=4, space="PSUM"))
        ps_log = ctx.enter_context(tc.tile_pool(name="ps_log", bufs=1, space="PSUM"))
        ps_y = ctx.enter_context(tc.tile_pool(name="ps_y", bufs=3, space="PSUM"))

        # ---- persistent loads, just-in-time order across both queues ----
        xT_sb = persist.tile([128, KD, GRID], BF, tag="xT")
        w1a_sb = persist.tile([128, KD, H], BF, tag="w1a")
        w1c_sb = persist.tile([128, KD, H], BF, tag="w1c")
        w2rep_sb = persist.tile([128, MH, 128], BF, tag="w2rep")
        wv_sb = persist.tile([128, KD, D], BF, tag="wv")
        m1_sb = persist.tile([128, KD, D], BF, tag="m1")
        m2_sb = persist.tile([128, KD, D], BF, tag="m2")
        b1_sb = persist.tile([128, MH], FP32, tag="b1")
        wvb_sb = persist.tile([128, MD], FP32, tag="wvb")
        b2_sb = persist.tile([128, 1], FP32, tag="b2")
        xT_r = xT.rearrange("(n p) t -> p n t", p=128)
        w1a_r = w1a.rearrange("(n p) m -> p n m", p=128)
        w1c_r = w1c.rearrange("(n p) m -> p n m", p=128)
        w2_r = w2rep.rearrange("(n p) m -> p n m", p=128)
        wv_r = wv.rearrange("(n p) m -> p n m", p=128)
        m1_r = m1.rearrange("(n p) m -> p n m", p=128)
        m2_r = m2.rearrange("(n p) m -> p n m", p=128)
        Q1 = QT + HALO
        # Greedy per-queue byte balancing in PE-consumption order:
        #   sync:   xTq0 | w1c | wv[0:4] | xTq3 | m1
        #   scalar: w1a | xTq1 | wv[4:8] | xTq2 | w2rep+biases | m2
        nc.sync.dma_start(out=xT_sb[:, :, 0:Q1], in_=xT_r[:, :, 0:Q1])
        for mc in range(MH):  # w1a col-chunks so a(q0, m=0) unblocks early
            nc.scalar.dma_start(
                out=w1a_sb[:, :, mc * 128:(mc + 1) * 128],
                in_=w1a_r[:, :, mc * 128:(mc + 1) * 128],
            )
        nc.scalar.dma_start(out=b1_sb, in_=b1r[:, :])
        for mc in range(MH):
            nc.sync.dma_start(
                out=w1c_sb[:, :, mc * 128:(mc + 1) * 128],
                in_=w1c_r[:, :, mc * 128:(mc + 1) * 128],
            )
        nc.scalar.dma_start(
            out=xT_sb[:, :, Q1:Q1 + QT], in_=xT_r[:, :, Q1:Q1 + QT]
        )
        nc.scalar.dma_start(out=wvb_sb, in_=wvbr[:, :])
        for mc in range(MD):
            eng = nc.sync if mc < 4 else nc.scalar
            eng.dma_start(
                out=wv_sb[:, :, mc * 128:(mc + 1) * 128],
                in_=wv_r[:, :, mc * 128:(mc + 1) * 128],
            )
        nc.scalar.dma_start(
            out=xT_sb[:, :, Q1 + QT:Q1 + 2 * QT],
            in_=xT_r[:, :, Q1 + QT:Q1 + 2 * QT],
        )
        nc.sync.dma_start(
            out=xT_sb[:, :, Q1 + 2 * QT:GRID], in_=xT_r[:, :, Q1 + 2 * QT:GRID]
        )
        nc.scalar.dma_start(out=w2rep_sb[:, :, :], in_=w2_r[:, :, :])
        nc.scalar.dma_start(out=b2_sb, in_=b2r[:, :])
        # merge weights last (first needed after phases A+B)
        nc.sync.dma_start(out=m1_sb[:, :, 0:512], in_=m1_r[:, :, 0:512])
        nc.scalar.dma_start(out=m2_sb[:, :, 0:512], in_=m2_r[:, :, 0:512])
        nc.sync.dma_start(out=m1_sb[:, :, 512:D], in_=m1_r[:, :, 512:D])
        nc.scalar.dma_start(out=m2_sb[:, :, 512:D], in_=m2_r[:, :, 512:D])
        magic_sb = persist.tile([128, 1], I32, tag="magic")
        nc.vector.memset(magic_sb, 0x5F3759DF)
        one_i = persist.tile([128, 1], I32, tag="onei")
        nc.vector.memset(one_i, 1)
        if use_gamma_beta:
            gam_sb = persist.tile([128, D], FP32, tag="gam")
            nc.sync.dma_start(out=gam_sb, in_=gam.partition_broadcast(128))
            bet_sb = persist.tile([128, D], FP32, tag="bet")
            nc.sync.dma_start(out=bet_sb, in_=bet.partition_broadcast(128))
        if use_merge_b:
            mb_sb = persist.tile([128, D], FP32, tag="mb")
            nc.sync.dma_start(out=mb_sb, in_=mbt.partition_broadcast(128))

        # ---- phase A: a (own grid) and c (shifted grid), all quarters ----
        aqs, cqs, vzqs = [], [], []
        for q in range(NQ):
            g0 = q * QT
            aq = abpool.tile([128, MH, QT], BF, tag="aq")
            aqs.append(aq)
            cq = abpool.tile([128, MH, QG], BF, tag="cq")
            cqs.append(cq)
            for m in range(MH):
                ps = ps_acc.tile([128, QT], FP32, tag="acc")
                for k in range(KD):
                    nc.tensor.matmul(
                        ps,
                        w1a_sb[:, k, m * 128:(m + 1) * 128],
                        xT_sb[:, k, g0 + HALO:g0 + HALO + QT],
                        start=(k == 0),
                        stop=(k == KD - 1),
                    )
                nc.scalar.activation(
                    out=aq[:, m, :], in_=ps, func=AF.Identity,
                    bias=b1_sb[:, m:m + 1], scale=1.0,
                )
            for m in range(MH):
                ps = ps_acc.tile([128, QG], FP32, tag="acc")
                for k in range(KD):
                    nc.tensor.matmul(
                        ps,
                        w1c_sb[:, k, m * 128:(m + 1) * 128],
                        xT_sb[:, k, g0:g0 + QG],
                        start=(k == 0),
                        stop=(k == KD - 1),
                    )
                nc.scalar.copy(out=cq[:, m, :], in_=ps)
        # ---- phase B: vzb (shifted grid), all quarters -------------------
        for q in range(NQ):
            g0 = q * QT
            vzq = abpool.tile([128, MD, QG], BF, tag="vzq")
            vzqs.append(vzq)
            for m in range(MD):
                ps = ps_acc.tile([128, QG], FP32, tag="acc")
                for k in range(KD):
                    nc.tensor.matmul(
                        ps,
                        wv_sb[:, k, m * 128:(m + 1) * 128],
                        xT_sb[:, k, g0:g0 + QG],
                        start=(k == 0),
                        stop=(k == KD - 1),
                    )
                nc.scalar.activation(
                    out=vzq[:, m, :], in_=ps, func=AF.Identity,
                    bias=wvb_sb[:, m:m + 1], scale=1.0,
                )
        # ---- phase C: gating (hid -> tau -> msg), per quarter ------------
        # silu(z) = z * sigmoid(z) keeps ScalarE in one activation-table set
        # for the whole kernel (silu/sqrt live in different sets; a switch
        # costs ~2.7us). tau comes out of its matmul pre-broadcast across
        # partitions because w2 is replicated over all 128 PE columns.
        msgqs = []
        for q in range(NQ):
            aq, cq, vzq = aqs[q], cqs[q], vzqs[q]
            tauq = qpool.tile([128, W, QT], BF, tag="tauq")
            for p in range(W // 2):
                hs = mpool2.tile([128, MH, 2, QT], BF, tag="hs")
                for wi in range(2):
                    w = 2 * p + wi
                    o = HALO - 1 - w
                    nc.vector.tensor_add(
                        hs[:, :, wi, :], aq, cq[:, :, o:o + QT]
                    )
                sg = mpool2.tile([128, MH, 2, QT], BF, tag="sg")
                nc.scalar.activation(out=sg, in_=hs, func=AF.Sigmoid)
                hss = mpool2.tile([128, MH, 2, QT], BF, tag="hids")
                nc.vector.tensor_mul(hss, hs, sg)
                pl = ps_log.tile([128, 2 * QT], FP32, tag="logit")
                for k in range(MH):
                    nc.tensor.matmul(
                        pl,
                        w2rep_sb[:, k, :],
                        hss[:, k, :, :],
                        start=(k == 0),
                        stop=(k == MH - 1),
                    )
                nc.scalar.activation(
                    out=tauq[:, 2 * p:2 * p + 2, :],
                    in_=pl.rearrange("p (a b) -> p a b", a=2),
                    func=AF.Sigmoid,
                    bias=b2_sb[:, 0:1], scale=1.0,
                )
            # msg = sum_w tau_w * shift(vzb, w+1): fused 3D bf16 ops with tau
            # broadcast over the 8 d-tiles via a step-0 mid dimension.
            msgq = qpool.tile([128, MD, QT], BF, tag="msgq")
            msgqs.append(msgq)

            def tau_b(w, tauq=tauq):
                s = tauq[:, w, :]
                return bass.AP(
                    tensor=s.tensor, offset=s.offset,
                    ap=[s.ap[0], [0, MD], s.ap[1]],
                )

            pw = []
            for w in range(W):
                o = HALO - 1 - w
                pt = mpool.tile([128, MD, QT], BF, tag="pw")
                nc.vector.tensor_mul(pt, tau_b(w), vzq[:, :, o:o + QT])
                pw.append(pt)
                if w == 1:
                    m01 = mpool.tile([128, MD, QT], BF, tag="pw")
                    nc.vector.tensor_add(m01, pw[0], pw[1])
            nc.vector.tensor_add(pw[3], pw[2], pw[3])
            nc.vector.tensor_add(msgq, m01, pw[3])
        # ---- phase D: merge + LayerNorm + store, per quarter -------------
        for q in range(NQ):
            g0 = q * QT
            msgq = msgqs[q]
            srow = mpool.tile([128, NT, 2], FP32, tag="srow")
            sqs = mpool.tile([128, NT, 2], FP32, tag="sqs")
            ysb = []
            for tt in range(NT):
                tok0 = g0 + 128 * tt
                yt = opool.tile([128, D], FP32, tag="ysb")
                ysb.append(yt)
                for half in range(2):
                    n0 = half * 512
                    yps = ps_y.tile([128, 512], FP32, tag="y")
                    for k in range(KD):
                        nc.tensor.matmul(
                            yps,
                            xT_sb[:, k, HALO + tok0:HALO + tok0 + 128],
                            m1_sb[:, k, n0:n0 + 512],
                            start=(k == 0),
                            stop=False,
                        )
                    for k in range(KD):
                        nc.tensor.matmul(
                            yps,
                            msgq[:, k, 128 * tt:128 * tt + 128],
                            m2_sb[:, k, n0:n0 + 512],
                            start=False,
                            stop=(k == KD - 1),
                        )
                    if use_merge_b:
                        nc.vector.tensor_add(yps, yps, mb_sb[:, n0:n0 + 512])
                    # Evict PSUM while collecting LN stats: Copy gives sum(y),
                    # Square gives sum(y^2) — both stay in the sigmoid table
                    # set. 'junk' is a write-only sink for the Square pass.
                    nc.scalar.activation(
                        out=yt[:, n0:n0 + 512], in_=yps, func=AF.Copy,
                        accum_out=srow[:, tt, half:half + 1],
                    )
                    junk = mpool2.tile([128, 512], FP32, tag="junk")
                    nc.scalar.activation(
                        out=junk, in_=yps, func=AF.Square,
                        accum_out=sqs[:, tt, half:half + 1],
                    )
            # LayerNorm finalize for both token tiles at once; rstd via
            # bit-trick seed + 2 Newton steps (keeps sqrt off ScalarE).
            ssum = mpool.tile([128, NT], FP32, tag="ssum")
            nc.vector.reduce_sum(out=ssum, in_=srow, axis=AX.X)
            qsum = mpool.tile([128, NT], FP32, tag="qsum")
            nc.vector.reduce_sum(out=qsum, in_=sqs, axis=AX.X)
            mean = mpool.tile([128, NT], FP32, tag="mean")
            nc.vector.tensor_scalar_mul(mean, ssum, 1.0 / D)
            m2e = mpool.tile([128, NT], FP32, tag="m2e")
            nc.vector.scalar_tensor_tensor(   # mean^2 - eps
                out=m2e, in0=mean, scalar=1.0, in1=mean,
                op0=ALU.mult, op1=ALU.mult,
            )
            nc.vector.tensor_scalar_add(m2e, m2e, -EPS)
            veps = mpool.tile([128, NT], FP32, tag="veps")
            nc.vector.scalar_tensor_tensor(   # q/D - (mean^2 - eps)
                out=veps, in0=qsum, scalar=1.0 / D, in1=m2e,
                op0=ALU.mult, op1=ALU.subtract,
            )
            rbits = mpool.tile([128, NT], I32, tag="rbits")
            nc.vector.tensor_scalar(
                out=rbits, in0=veps.bitcast(I32), scalar1=one_i[:, 0:1],
                scalar2=None, op0=ALU.arith_shift_right,
            )
            nc.vector.tensor_tensor(
                out=rbits, in0=magic_sb.to_broadcast([128, NT]), in1=rbits,
                op=ALU.subtract,
            )
            rstd = rbits.bitcast(FP32)
            for _ in range(2):
                nt1 = mpool.tile([128, NT], FP32, tag="nt1")
                nc.vector.tensor_mul(nt1, rstd, rstd)
                nc.vector.tensor_mul(nt1, nt1, veps)
                nc.vector.tensor_scalar(
                    out=nt1, in0=nt1, scalar1=-0.5, scalar2=1.5,
                    op0=ALU.mult, op1=ALU.add,
                )
                nc.vector.tensor_mul(rstd, rstd, nt1)
            for tt in range(NT):
                tok0 = g0 + 128 * tt
                nc.vector.tensor_scalar(
                    out=ysb[tt], in0=ysb[tt], scalar1=mean[:, tt:tt + 1],
                    scalar2=rstd[:, tt:tt + 1],
                    op0=ALU.subtract, op1=ALU.mult,
                )
                if use_gamma_beta:
                    nc.vector.tensor_mul(ysb[tt], ysb[tt], gam_sb)
                    nc.vector.tensor_add(ysb[tt], ysb[tt], bet_sb)
                nc.sync.dma_start(out=y[tok0:tok0 + 128, :], in_=ysb[tt])
    nc.compile()
    return nc


_CACHE: dict = {}


def _get_nc(use_gamma_beta: bool, use_merge_b: bool):
    key = (use_gamma_beta, use_merge_b)
    if key not in _CACHE:
        _CACHE[key] = build_nc(use_gamma_beta, use_merge_b)
    return _CACHE[key]


def kernel(x, w1, b1, w2, b2, wv_w, wv_b, merge_w, merge_b, gamma, beta):
    x = np.asarray(x, dtype=np.float32)
    w1 = np.asarray(w1, dtype=np.float32)
    b1 = np.asarray(b1, dtype=np.float32)
    w2 = np.asarray(w2, dtype=np.float32)
    b2 = np.asarray(b2, dtype=np.float32)
    wv_w = np.asarray(wv_w, dtype=np.float32)
    wv_b = np.asarray(wv_b, dtype=np.float32)
    merge_w = np.asarray(merge_w, dtype=np.float32)
    merge_b = np.asarray(merge_b, dtype=np.float32)
    gamma = np.asarray(gamma, dtype=np.float32)
    beta = np.asarray(beta, dtype=np.float32)

    use_gamma_beta = not (np.all(gamma == 1.0) and np.all(beta == 0.0))
    use_merge_b = bool(np.any(merge_b != 0.0))
    nc = _get_nc(use_gamma_beta, use_merge_b)

    x2 = x.reshape(B * T, D)
    shared = {
        "w1a": w1[:D].astype(BF16),
        "w1c": w1[D:].astype(BF16),
        "wv": wv_w.astype(BF16),
        "m1": merge_w[:D].astype(BF16),
        "m2": merge_w[D:].astype(BF16),
        "w2rep": np.ascontiguousarray(
            np.broadcast_to(w2.reshape(H, 1), (H, 128))
        ).astype(BF16),
        "b1r": np.ascontiguousarray(b1.reshape(MH, 128).T),
        "wvbr": np.ascontiguousarray(wv_b.reshape(MD, 128).T),
        "b2r": np.full((128, 1), float(b2[0]), np.float32),
    }
    if use_gamma_beta:
        shared["gam"] = gamma.reshape(1, D)
        shared["bet"] = beta.reshape(1, D)
    if use_merge_b:
        shared["mbt"] = merge_b.reshape(1, D)

    in_maps = []
    for c in range(NCORES):
        t0 = c * NTOK
        xs = np.zeros((GRID, D), np.float32)
        xs[HALO:] = x2[t0:t0 + NTOK]
        if t0 % T != 0:  # halo stays inside the same batch element
            xs[:HALO] = x2[t0 - HALO:t0]
        m = dict(shared)
        m["xT"] = np.ascontiguousarray(xs.T).astype(BF16)
        in_maps.append(m)

    res = run_bass_kernel_spmd(nc, in_maps, core_ids=list(range(NCORES)))
    out = np.concatenate([r["y"] for r in res.results], axis=0)
    return out.reshape(B, T, D).astype(np.float32)



# revision 3
# speedup vs baseline: 1.4290x; 1.4290x over previous
"""Trainium2 Bass kernel for CausalTensionGraphLayer.

Math (reference factorization, with the wv/m2 merge folded on host):
  ac  = x @ [w1a | w1c] + [b1 | 0]      [grid, D]   (chunks 0-3 = a, 4-7 = c)
  u   = x @ P + qb                      [grid, D]   P = wv_w @ merge_w[D:],
                                                    qb = wv_b @ merge_w[D:]
  hid_w  = silu(a[t] + c[t-w-1])                    (c is 0 for t-w-1 < 0)
  tau_w  = sigmoid(hid_w @ w2 + b2)
  y[t]   = x[t] @ m1 + sum_w tau_w[t] * u[t-w-1] + merge_b
  out    = LayerNorm(y) * gamma + beta

The key identity: msg @ m2 = sum_w tau_w * (vzb @ m2) because tau_w[t] is a
per-token scalar, so the wv and merge projections collapse into one (P),
removing a full D x D matmul phase.  Zero halo rows of x reproduce the
out-of-range behaviour exactly (u = qb there, matching the wv-bias rule).

Sharding: data-parallel over B*T = 8192 token rows, 1024 own tokens per core
plus a 4-row causal halo (zeros at batch boundaries).  No collectives.  All
weights and x are pre-cast to bf16 and pre-arranged on host into the exact
SBUF tile layouts so every input DMA is one large contiguous-per-partition
transfer.

Schedule per token quarter q: A(q) = ac matmuls, the gating front (hs/sigmoid)
is emitted between A and U so the scalar sigmoid overlaps U's matmuls, U(q) =
u matmuls, then tau matmul + gate-sum gs (DVE).  After all quarters, D(q)
computes y = x@m1 into PSUM and accumulates gs via 128x128 identity-matmul
transposes into the same banks, evicts with fused LayerNorm statistics, and
stores bf16.  8 warm-up matmuls at t=0 lift the PE clock gate (HAM) to full
rate before the first real matmul.
"""

from contextlib import ExitStack

import numpy as np
import ml_dtypes

import concourse.bass as bass
import concourse.bacc as bacc
import concourse.tile as tile
from concourse import mybir
from concourse.bass_utils import run_bass_kernel_spmd

BF16 = ml_dtypes.bfloat16

B, T, D = 2, 4096, 1024
H = D // 2
W = 4
EPS = 1e-5
NCORES = 8
NTOK = (B * T) // NCORES          # 1024 own tokens per core
HALO = W                          # 4
GRID = NTOK + HALO                # 1028
NQ = 4                            # token quarters per core
QT = NTOK // NQ                   # 256 own tokens per quarter
QG = QT + HALO                    # 260 grid cols per quarter
KD = D // 128                     # 8 K-chunks over D
MH = H // 128                     # 4 M-tiles over H
MD = D // 128                     # 8 M-tiles over D
NT = QT // 128                    # 2 token tiles per quarter

FP32 = mybir.dt.float32
I32 = mybir.dt.int32
BF = mybir.dt.bfloat16
AF = mybir.ActivationFunctionType
ALU = mybir.AluOpType
AX = mybir.AxisListType


def build_nc(use_gamma_beta: bool, use_merge_b: bool):
    nc = bacc.Bacc(None, target_bir_lowering=False)

    xq = nc.dram_tensor("xq", [128, NQ, KD, QG], BF, kind="ExternalInput")
    wac = nc.dram_tensor("wac", [128, MD, KD, 128], BF, kind="ExternalInput")
    pw = nc.dram_tensor("pw", [128, MD, KD, 128], BF, kind="ExternalInput")
    m1 = nc.dram_tensor("m1", [128, 2, KD, 512], BF, kind="ExternalInput")
    w2rep = nc.dram_tensor("w2rep", [128, MH, 128], BF, kind="ExternalInput")
    iden = nc.dram_tensor("iden", [128, 128], BF, kind="ExternalInput")
    b1z = nc.dram_tensor("b1z", [128, MD], FP32, kind="ExternalInput")
    qbr = nc.dram_tensor("qbr", [128, MD], FP32, kind="ExternalInput")
    b2r = nc.dram_tensor("b2r", [128, 1], FP32, kind="ExternalInput")
    if use_gamma_beta:
        gam = nc.dram_tensor("gam", [1, D], FP32, kind="ExternalInput")
        bet = nc.dram_tensor("bet", [1, D], FP32, kind="ExternalInput")
    if use_merge_b:
        mbt = nc.dram_tensor("mbt", [1, D], FP32, kind="ExternalInput")
    y = nc.dram_tensor("y", [NTOK, D], BF, kind="ExternalOutput")

    with tile.TileContext(nc) as tc, ExitStack() as ctx:
        persist = ctx.enter_context(tc.tile_pool(name="persist", bufs=1))
        acpool = ctx.enter_context(tc.tile_pool(name="acpool", bufs=2))
        gspool = ctx.enter_context(tc.tile_pool(name="gspool", bufs=NQ))
        mpool = ctx.enter_context(tc.tile_pool(name="mpool", bufs=2))
        opool = ctx.enter_context(tc.tile_pool(name="opool", bufs=3))
        ps_acc = ctx.enter_context(tc.tile_pool(name="ps_acc", bufs=3, space="PSUM"))
        ps_log = ctx.enter_context(tc.tile_pool(name="ps_log", bufs=1, space="PSUM"))
        ps_y = ctx.enter_context(tc.tile_pool(name="ps_y", bufs=3, space="PSUM"))

        # ---- persistent tiles (SBUF layouts match DRAM exactly) ----------
        xq_sb = persist.tile([128, NQ, KD, QG], BF, tag="xq")
        wac_sb = persist.tile([128, MD, KD, 128], BF, tag="wac")
        pw_sb = persist.tile([128, MD, KD, 128], BF, tag="pw")
        m1_sb = persist.tile([128, 2, KD, 512], BF, tag="m1")
        w2rep_sb = persist.tile([128, MH, 128], BF, tag="w2rep")
        iden_sb = persist.tile([128, 128], BF, tag="iden")
        b1z_sb = persist.tile([128, MD], FP32, tag="b1z")
        qb_sb = persist.tile([128, MD], FP32, tag="qb")
        b2_sb = persist.tile([128, 1], FP32, tag="b2")

        # Input DMAs, split across the two HWDGE rings (sync/scalar) in the
        # exact order the PE consumes them.
        for mc in range(4):
            nc.sync.dma_start(
                out=wac_sb[:, 2 * mc:2 * mc + 2], in_=wac[:, 2 * mc:2 * mc + 2]
            )
        nc.scalar.dma_start(out=xq_sb[:, 0], in_=xq[:, 0])
        nc.scalar.dma_start(out=pw_sb[:, 0:2], in_=pw[:, 0:2])
        nc.scalar.dma_start(out=pw_sb[:, 2:4], in_=pw[:, 2:4])
        nc.scalar.dma_start(out=w2rep_sb, in_=w2rep[:, :])
        nc.scalar.dma_start(out=b1z_sb, in_=b1z[:, :])
        nc.scalar.dma_start(out=qb_sb, in_=qbr[:, :])
        nc.scalar.dma_start(out=b2_sb, in_=b2r[:, :])
        nc.scalar.dma_start(out=iden_sb, in_=iden[:, :])
        if use_gamma_beta:
            gam_sb = persist.tile([128, D], FP32, tag="gam")
            nc.scalar.dma_start(out=gam_sb, in_=gam.partition_broadcast(128))
            bet_sb = persist.tile([128, D], FP32, tag="bet")
            nc.scalar.dma_start(out=bet_sb, in_=bet.partition_broadcast(128))
        if use_merge_b:
            mb_sb = persist.tile([128, D], FP32, tag="mb")
            nc.scalar.dma_start(out=mb_sb, in_=mbt.partition_broadcast(128))
        nc.scalar.dma_start(out=pw_sb[:, 4:6], in_=pw[:, 4:6])
        nc.scalar.dma_start(out=pw_sb[:, 6:8], in_=pw[:, 6:8])
        nc.sync.dma_start(out=xq_sb[:, 1], in_=xq[:, 1])
        nc.sync.dma_start(out=xq_sb[:, 2], in_=xq[:, 2])
        nc.scalar.dma_start(out=xq_sb[:, 3], in_=xq[:, 3])
        nc.sync.dma_start(out=m1_sb[:, 0], in_=m1[:, 0])
        nc.scalar.dma_start(out=m1_sb[:, 1], in_=m1[:, 1])

        magic_sb = persist.tile([128, 1], I32, tag="magic")
        nc.vector.memset(magic_sb, 0x5F3759DF)
        one_i = persist.tile([128, 1], I32, tag="onei")
        nc.vector.memset(one_i, 1)

        # ---- HAM warm-up: ~3.4us of dummy matmuls while inputs stream ----
        warm_sb = persist.tile([128, 512], BF, tag="warm")
        nc.vector.memset(warm_sb, 0)
        warm_ps = ps_acc.tile([128, 512], FP32, tag="warm", bufs=1)
        NWARM = 8
        for i in range(NWARM):
            nc.tensor.matmul(
                warm_ps, warm_sb[:, 0:128], warm_sb,
                start=(i == 0), stop=(i == NWARM - 1),
            )

        # ---- main per-quarter pipeline -----------------------------------
        def tau_b(tauq, w):
            s = tauq[:, w, :]
            return bass.AP(
                tensor=s.tensor, offset=s.offset,
                ap=[s.ap[0], [0, MD], s.ap[1]],
            )

        acqs, uqs, gsqs = [], [], []
        for q in range(NQ):
            # A(q): ac = x @ [w1a|w1c] (+ [b1|0]) on the quarter grid
            acq = acpool.tile([128, MD, QG], BF, tag="acq")
            acqs.append(acq)
            for m in range(MD):
                ps = ps_acc.tile([128, QG], FP32, tag="acc")
                for k in range(KD):
                    nc.tensor.matmul(
                        ps,
                        wac_sb[:, m, k, :],
                        xq_sb[:, q, k, :],
                        start=(k == 0),
                        stop=(k == KD - 1),
                    )
                nc.vector.tensor_scalar_add(acq[:, m, :], ps, b1z_sb[:, m:m + 1])
            # gating front: hs = a + shift(c), sigmoid on ScalarE (overlaps U)
            hss2 = []
            for p in range(W // 2):
                hs = mpool.tile([128, MH, 2, QT], BF, tag="hs")
                for wi in range(2):
                    w = 2 * p + wi
                    o = HALO - 1 - w
                    nc.vector.tensor_add(
                        hs[:, :, wi, :],
                        acq[:, 0:MH, HALO:HALO + QT],
                        acq[:, MH:MD, o:o + QT],
                    )
                sg = mpool.tile([128, MH, 2, QT], BF, tag="sg")
                nc.scalar.activation(out=sg, in_=hs, func=AF.Sigmoid)
                hss2.append((hs, sg))
            # U(q): u = x @ P + qb on the quarter grid
            uq = acpool.tile([128, MD, QG], BF, tag="uq")
            uqs.append(uq)
            for m in range(MD):
                ps = ps_acc.tile([128, QG], FP32, tag="acc")
                for k in range(KD):
                    nc.tensor.matmul(
                        ps,
                        pw_sb[:, m, k, :],
                        xq_sb[:, q, k, :],
                        start=(k == 0),
                        stop=(k == KD - 1),
                    )
                nc.vector.tensor_scalar_add(uq[:, m, :], ps, qb_sb[:, m:m + 1])
            # tau: silu = hs * sigmoid(hs), tiny matmul vs replicated w2
            tauq = mpool.tile([128, W, QT], BF, tag="tauq")
            for p in range(W // 2):
                hs, sg = hss2[p]
                hss = mpool.tile([128, MH, 2, QT], BF, tag="hss")
                nc.vector.tensor_mul(hss, hs, sg)
                pl = ps_log.tile([128, 2 * QT], FP32, tag="logit")
                for k in range(MH):
                    nc.tensor.matmul(
                        pl,
                        w2rep_sb[:, k, :],
                        hss[:, k, :, :],
                        start=(k == 0),
                        stop=(k == MH - 1),
                    )
                nc.scalar.activation(
                    out=tauq[:, 2 * p:2 * p + 2, :],
                    in_=pl.rearrange("p (a b) -> p a b", a=2),
                    func=AF.Sigmoid,
                    bias=b2_sb[:, 0:1], scale=1.0,
                )
            # gate-sum gs = sum_w tau_w * shift(u, w+1), bf16 on DVE
            gsq = gspool.tile([128, MD, QT], BF, tag="gsq")
            gsqs.append(gsq)
            pw_t = []
            for w in range(W):
                o = HALO - 1 - w
                pt = mpool.tile([128, MD, QT], BF, tag="pw", bufs=4)
                nc.vector.tensor_mul(pt, tau_b(tauq, w), uq[:, :, o:o + QT])
                pw_t.append(pt)
                if w == 1:
                    m01 = mpool.tile([128, MD, QT], BF, tag="pw", bufs=4)
                    nc.vector.tensor_add(m01, pw_t[0], pw_t[1])
            nc.vector.tensor_add(pw_t[3], pw_t[2], pw_t[3])
            nc.vector.tensor_add(gsq, m01, pw_t[3])

        # ---- phase D: merge + transpose-accumulate + LayerNorm + store ---
        for q in range(NQ):
            g0 = q * QT
            gsq = gsqs[q]
            srow = mpool.tile([128, NT, 2], FP32, tag="srow")
            sqs = mpool.tile([128, NT, 2], FP32, tag="sqs")
            ysb = []
            for tt in range(NT):
                yt = opool.tile([128, D], FP32, tag="ysb")
                ysb.append(yt)
                for half in range(2):
                    n0 = half * 512
                    yps = ps_y.tile([128, 512], FP32, tag="y")
                    for k in range(KD):
                        nc.tensor.matmul(
                            yps,
                            xq_sb[:, q, k, HALO + 128 * tt:HALO + 128 * tt + 128],
                            m1_sb[:, half, k, :],
                            start=(k == 0),
                            stop=False,
                        )
                    # gs arrives transposed via identity matmuls, accumulated
                    # into the same bank (tau-gated message + x@m1 in one go).
                    for mm in range(4):
                        m = half * 4 + mm
                        nc.tensor.matmul(
                            yps[:, mm * 128:(mm + 1) * 128],
                            gsq[:, m, 128 * tt:128 * tt + 128],
                            iden_sb,
                            start=False,
                            stop=(mm == 3),
                        )
                    if use_merge_b:
                        nc.vector.tensor_add(yps, yps, mb_sb[:, n0:n0 + 512])
                    nc.scalar.activation(
                        out=yt[:, n0:n0 + 512], in_=yps, func=AF.Copy,
                        accum_out=srow[:, tt, half:half + 1],
                    )
                    junk = mpool.tile([128, 512], FP32, tag="junk")
                    nc.scalar.activation(
                        out=junk, in_=yps, func=AF.Square,
                        accum_out=sqs[:, tt, half:half + 1],
                    )
            # LayerNorm finalize; rstd via bit-trick seed + 2 Newton steps
            # (keeps sqrt off ScalarE so one activation table serves all).
            ssum = mpool.tile([128, NT], FP32, tag="ssum")
            nc.vector.reduce_sum(out=ssum, in_=srow, axis=AX.X)
            qsum = mpool.tile([128, NT], FP32, tag="qsum")
            nc.vector.reduce_sum(out=qsum, in_=sqs, axis=AX.X)
            mean = mpool.tile([128, NT], FP32, tag="mean")
            nc.vector.tensor_scalar_mul(mean, ssum, 1.0 / D)
            m2e = mpool.tile([128, NT], FP32, tag="m2e")
            nc.vector.scalar_tensor_tensor(   # mean^2 - eps
                out=m2e, in0=mean, scalar=1.0, in1=mean,
                op0=ALU.mult, op1=ALU.mult,
            )
            nc.vector.tensor_scalar_add(m2e, m2e, -EPS)
            veps = mpool.tile([128, NT], FP32, tag="veps")
            nc.vector.scalar_tensor_tensor(   # q/D - (mean^2 - eps)
                out=veps, in0=qsum, scalar=1.0 / D, in1=m2e,
                op0=ALU.mult, op1=ALU.subtract,
            )
            rbits = mpool.tile([128, NT], I32, tag="rbits")
            nc.vector.tensor_scalar(
                out=rbits, in0=veps.bitcast(I32), scalar1=one_i[:, 0:1],
                scalar2=None, op0=ALU.arith_shift_right,
            )
            nc.vector.tensor_tensor(
                out=rbits, in0=magic_sb.to_broadcast([128, NT]), in1=rbits,
                op=ALU.subtract,
            )
            rstd = rbits.bitcast(FP32)
            for _ in range(2):
                nt1 = mpool.tile([128, NT], FP32, tag="nt1")
                nc.vector.tensor_mul(nt1, rstd, rstd)
                nc.vector.tensor_mul(nt1, nt1, veps)
                nc.vector.tensor_scalar(
                    out=nt1, in0=nt1, scalar1=-0.5, scalar2=1.5,
                    op0=ALU.mult, op1=ALU.add,
                )
                nc.vector.tensor_mul(rstd, rstd, nt1)
            for tt in range(NT):
                tok0 = g0 + 128 * tt
                ybf = opool.tile([128, D], BF, tag="ybf")
                if use_gamma_beta:
                    nc.vector.tensor_scalar(
                        out=ysb[tt], in0=ysb[tt], scalar1=mean[:, tt:tt + 1],
                        scalar2=rstd[:, tt:tt + 1],
                        op0=ALU.subtract, op1=ALU.mult,
                    )
                    nc.vector.tensor_mul(ysb[tt], ysb[tt], gam_sb)
                    nc.vector.tensor_add(ybf, ysb[tt], bet_sb)
                else:
                    nc.vector.tensor_scalar(
                        out=ybf, in0=ysb[tt], scalar1=mean[:, tt:tt + 1],
                        scalar2=rstd[:, tt:tt + 1],
                        op0=ALU.subtract, op1=ALU.mult,
                    )
                nc.sync.dma_start(out=y[tok0:tok0 + 128, :], in_=ybf)
    nc.compile()
    return nc


_CACHE: dict = {}


def _get_nc(use_gamma_beta: bool, use_merge_b: bool):
    key = (use_gamma_beta, use_merge_b)
    if key not in _CACHE:
        _CACHE[key] = build_nc(use_gamma_beta, use_merge_b)
    return _CACHE[key]


def kernel(x, w1, b1, w2, b2, wv_w, wv_b, merge_w, merge_b, gamma, beta):
    x = np.asarray(x, dtype=np.float32)
    w1 = np.asarray(w1, dtype=np.float32)
    b1 = np.asarray(b1, dtype=np.float32)
    w2 = np.asarray(w2, dtype=np.float32)
    b2 = np.asarray(b2, dtype=np.float32)
    wv_w = np.asarray(wv_w, dtype=np.float32)
    wv_b = np.asarray(wv_b, dtype=np.float32)
    merge_w = np.asarray(merge_w, dtype=np.float32)
    merge_b = np.asarray(merge_b, dtype=np.float32)
    gamma = np.asarray(gamma, dtype=np.float32)
    beta = np.asarray(beta, dtype=np.float32)

    use_gamma_beta = not (np.all(gamma == 1.0) and np.all(beta == 0.0))
    use_merge_b = bool(np.any(merge_b != 0.0))
    nc = _get_nc(use_gamma_beta, use_merge_b)

    m1f = merge_w[:D]
    m2f = merge_w[D:]
    P = wv_w @ m2f                          # fold wv and merge projections
    qb = wv_b @ m2f

    def wlayout(wmat):                      # [D, D] -> [128, MD, KD, 128]
        return np.ascontiguousarray(
            wmat.reshape(KD, 128, MD, 128).transpose(1, 2, 0, 3)
        ).astype(BF16)

    wac_h = wlayout(np.concatenate([w1[:D], w1[D:]], axis=1))
    pw_h = wlayout(P)
    m1_h = np.ascontiguousarray(
        m1f.reshape(KD, 128, 2, 512).transpose(1, 2, 0, 3)
    ).astype(BF16)
    w2_h = np.ascontiguousarray(
        np.broadcast_to(w2.reshape(MH, 128, 1), (MH, 128, 128)).transpose(1, 0, 2)
    ).astype(BF16)
    b1z = np.concatenate([b1, np.zeros(D - H, np.float32)])

    shared = {
        "wac": wac_h,
        "pw": pw_h,
        "m1": m1_h,
        "w2rep": w2_h,
        "iden": np.eye(128, dtype=BF16),
        "b1z": np.ascontiguousarray(b1z.reshape(MD, 128).T),
        "qbr": np.ascontiguousarray(qb.astype(np.float32).reshape(MD, 128).T),
        "b2r": np.full((128, 1), float(b2[0]), np.float32),
    }
    if use_gamma_beta:
        shared["gam"] = gamma.reshape(1, D)
        shared["bet"] = beta.reshape(1, D)
    if use_merge_b:
        shared["mbt"] = merge_b.reshape(1, D)

    x2T = np.ascontiguousarray(x.reshape(B * T, D).astype(BF16).T)  # [D, B*T]
    in_maps = []
    for c in range(NCORES):
        t0 = c * NTOK
        xsT = np.zeros((D, GRID), BF16)
        xsT[:, HALO:] = x2T[:, t0:t0 + NTOK]
        if t0 % T != 0:  # halo stays inside the same batch element
            xsT[:, :HALO] = x2T[:, t0 - HALO:t0]
        xk = xsT.reshape(KD, 128, GRID)
        xq_h = np.empty((128, NQ, KD, QG), BF16)
        for q in range(NQ):
            xq_h[:, q] = xk[:, :, q * QT:q * QT + QG].transpose(1, 0, 2)
        m = dict(shared)
        m["xq"] = xq_h
        in_maps.append(m)

    res = run_bass_kernel_spmd(nc, in_maps, core_ids=list(range(NCORES)))
    out = np.concatenate([r["y"] for r in res.results], axis=0)
    return out.reshape(B, T, D).astype(np.float32)
